# revision 76
# baseline (speedup 1.0000x reference)
"""Trainium2 Bass kernel for nn_Attention_53257594471037.

Multi-head attention layer (B=8, N=1024, embed 512 + class 512):
  qk = x[:, :, -512:] @ Wqk + bqk ; q, k = split(qk)      (8 heads, dh=64)
  v  = x @ Wv + bv                                        (8 heads, dv=128)
  out = softmax(q k^T / sqrt(64)) v                       per head
  y  = concat(out) @ Wo + bo
Sharding: data-parallel over batch - each of the 8 NeuronCores handles one
batch element end to end.  No collectives.

Key speed tricks vs a plain bf16 kernel:
  - The x / Wqk / Wv projections run as fp8e4m3 DoubleRow matmuls.  Pure
    fp8 is far too coarse (2.5-5% error vs the 2% budget), so each operand
    is split ON THE HOST into an fp8 hi + fp8 residual lo pair (hi+lo
    carries ~12 significant bits, better than bf16) and each product is
    computed with three DR terms: hi*hi (chunk-paired planes), plus
    (lo*hi + hi*lo) packed as the two planes of per-chunk DR matmuls.
    Per the calibrated cost model a DR matmul costs out_free x 0.5 PE
    cycles while contracting 2x128 rows, so the 3-term product runs at
    0.75x the bf16 cycle count with bf16-level accuracy.
  - x is also pre-transposed and pre-tiled on the host, so the kernel has
    no x transposes at all; Wqk columns are pre-permuted pair-major
    (q01,k01,q23,...) so attention pair 0 is gated by only 2 projection
    steps.
  - The y projection also runs as 3-term DR: Wo is host-split, and out^T
    is emitted as an on-chip fp8 hi/lo pair right at the out-transpose
    (hi = fp8(4*out^T) and lo = 4*out^T - hi, one DVE op each straight
    from the transpose PSUM).
  - Attention itself (scores, exp, PV) stays bf16: the error budget
    forbids pure fp8 there (each fp8 stage alone adds 2.4-5.5% vs the 2%
    budget), and for K=64 scores DR has no advantage.  Softmax uses the
    S^T layout (j on partitions) so exp runs straight out of PSUM on ACT;
    the denominator comes free from a ones column in v_aug; bv rides
    through attention (softmax rows sum to 1) and is folded into the PV
    normalization (one scalar_tensor_tensor per PV step).
  - A PE p-state warm-up (dummy matmuls during the initial DMA wait)
    makes all real work run at the full 2.4 GHz clock.

Emission order keeps the PE busy through the ACT-bound softmax middle:
pair 0 carries the remaining qk-projection steps, pairs 1-2 carry the
v-projection and PV of the previous pair plus the first out-transposes,
pair 3 carries PV-2 + out-transposes of heads 2-3, and the tail pipelines
PV-3 / out-transposes of heads 4-7 / y-projection per i-tile with the
y steps one tile behind so their operands are never on the critical path.
"""

import os

os.environ.setdefault("MYCRO_LOCAL_CACHE", "1")

import numpy as np
import ml_dtypes

E4NP = ml_dtypes.float8_e4m3
BFNP = ml_dtypes.bfloat16

# --- problem constants (hardcoded; kernel.py must be self-contained) ---
B = 8
N = 1024          # tokens
D = 1024          # embed + class feature width
CLS = 512         # class width; qk projection reads x[:, :, -CLS:]
HEADS = 8
DH = 64           # per-head q/k dim
DV = 128          # per-head v dim
SCALE = DH ** -0.5
NT = N // 128     # 8 token tiles
DC = D // 128     # 8 feature chunks
VSTRIDE = 130     # per-head stride in v_aug: 128 data + 1 ones + 1 pad
SX = 16.0         # host fp8 scale on x
SW = 32.0         # host fp8 scale on Wqk / Wv / Wo
SOUT = 4.0        # on-chip fp8 scale on out^T
INV = 1.0 / (SX * SW)
INV_Y = 1.0 / (SOUT * SW)

_COMPILED = None  # cached compiled module so repeated kernel() calls reuse it

# Wqk column-block permutation: pair-major (q01, k01, q23, k23, ...) so the
# first two projection steps alone gate attention pair 0.
_MPERM = [0, 4, 1, 5, 2, 6, 3, 7]


def _build():
    import concourse.mybir as mybir
    import concourse.tile as tile
    from concourse import bacc

    f32 = mybir.dt.float32
    bf16 = mybir.dt.bfloat16
    fp8 = mybir.dt.float8e4
    DR = mybir.MatmulPerfMode.DoubleRow
    Exp = mybir.ActivationFunctionType.Exp
    Ident = mybir.ActivationFunctionType.Identity
    mult = mybir.AluOpType.mult
    add = mybir.AluOpType.add
    subtract = mybir.AluOpType.subtract

    nc = bacc.Bacc(None, target_bir_lowering=False)

    # host-prepped layouts (see _prep_shared/_prep_x):
    #   xs  [128, 8, 2, 1024] fp8:    x^T tiled feat=(128c+p), plane0=hi, plane1=lo
    #   wqk [128, 8, 4, 2, 128] fp8:  m-block-major col-permuted Wqk, lo-first
    #   wv  [128, 8, 2, 1024] fp8:    plane0=lo, plane1=hi
    #   wo  [128, 8, 2, 1024] fp8:    plane0=lo, plane1=hi (vfeat=128c+p)
    xs_d = nc.declare_dram_parameter("xs", [128, DC, 2, N], fp8, isOutput=False)
    wqk_d = nc.declare_dram_parameter("wqk", [128, 8, 4, 2, 128], fp8, isOutput=False)
    wv_d = nc.declare_dram_parameter("wv", [128, DC, 2, 1024], fp8, isOutput=False)
    wo_d = nc.declare_dram_parameter("wo", [128, DC, 2, 1024], fp8, isOutput=False)
    bias_d = nc.declare_dram_parameter("biasc", [128, 16], f32, isOutput=False)
    # bob holds two partition-broadcast rows: [:, 0:1024] = bo, [:, 1024:2048] = bv
    bo_d = nc.declare_dram_parameter("bob", [128, 2 * D], bf16, isOutput=False)
    y_d = nc.declare_dram_parameter("y", [N, D], f32, isOutput=True)

    ident_const = nc.inline_tensor(
        np.eye(128, dtype=np.float32).astype(BFNP), name="identc"
    )

    with tile.TileContext(nc) as tc:
        with (
            tc.tile_pool(name="persist", bufs=1) as pp,
            tc.tile_pool(name="expsp", bufs=4) as ep,
            tc.tile_pool(name="small", bufs=2) as sp,
            tc.tile_pool(name="yout", bufs=4) as yp,
            tc.tile_pool(name="ps_mm", bufs=2, space="PSUM") as ps_mm,
            tc.tile_pool(name="ps_s", bufs=2, space="PSUM") as ps_s,
            tc.tile_pool(name="ps_o", bufs=2, space="PSUM") as ps_o,
        ):
            # ---------- loads (all HWDGE on the sync queue, no casts) -------
            # Order matters: the combined bias columns are tiny and gate the
            # psum->SBUF copies; wqk is m-block-major so the two blocks
            # (q01, k01) that gate attention pair 0 ride in a small first
            # DMA; the rest can trail in.
            wqk_sb = pp.tile([128, 8, 4, 2, 128], fp8)
            nc.sync.dma_start(out=wqk_sb[:, 0:2, :, :, :], in_=wqk_d[:, 0:2, :, :, :])
            x_sb = pp.tile([128, DC, 2, N], fp8, tag="xslot")
            nc.sync.dma_start(out=x_sb[:, 4:8, :, 0:512], in_=xs_d[:, 4:8, :, 0:512])
            bias_sb = pp.tile([128, 16], f32)
            nc.sync.dma_start(out=bias_sb[:, :], in_=bias_d[:, :])
            bqk_col = bias_sb[:, 0:8]
            bv_col = bias_sb[:, 8:16]
            nc.sync.dma_start(out=x_sb[:, 4:8, :, 512:N], in_=xs_d[:, 4:8, :, 512:N])
            nc.sync.dma_start(out=wqk_sb[:, 2:8, :, :, :], in_=wqk_d[:, 2:8, :, :, :])
            wv_sb = pp.tile([128, DC, 2, 1024], fp8)
            nc.sync.dma_start(out=wv_sb[:, :, :, :], in_=wv_d[:, :, :, :])
            nc.sync.dma_start(out=x_sb[:, 0:4, :, :], in_=xs_d[:, 0:4, :, :])
            ident = pp.tile([128, 128], bf16)
            nc.sync.dma_start(out=ident[:, :], in_=ident_const[:, :])
            wo_sb = pp.tile([128, DC, 2, 1024], fp8)
            nc.sync.dma_start(out=wo_sb[:, :, :, :], in_=wo_d[:, :, :, :])
            bob_sb = pp.tile([128, 2 * D], bf16)
            nc.sync.dma_start(out=bob_sb[:, :], in_=bo_d[:, :])
            bo_bc = bob_sb[:, 0:D]
            bv_bc = bob_sb[:, D : 2 * D]

            # ---------- PE p-state warm-up --------------------------------
            # The PE clock ramps 1.2 -> 2.4 GHz only after ~3us of sustained
            # activity.  While waiting for the x/Wqk DMAs, burn the idle
            # window on dummy transposes (fed by a memset tile so nothing
            # gates them) so the real work starts at full clock.
            wtile = pp.tile([128, 128], bf16, name="warmsrc")
            nc.vector.memset(wtile[:, :], 1.0)
            ps_w = ps_o.tile([128, 128], f32, tag="o", name="warm")
            for _ in range(37):
                nc.tensor.matmul(
                    ps_w[:, :], lhsT=wtile[:, :], rhs=wtile[:, :], start=True, stop=True
                )

            # ---------- qkT[f, n] = (Wqk^T @ x_clsT)/512 + bqk --------------
            # fp8 DoubleRow, 3-term residual: hi*hi over chunk pairs, then
            # (lo*hi + hi*lo) as the two planes of per-chunk DR matmuls.
            # m is pair-major: 2p = q of head pair p, 2p+1 = k of pair p.
            qkT = pp.tile([128, 8, N], bf16)

            def qkt_proj_step(m, nh, on_act=False, pool=None):
                ps = (pool or ps_mm).tile(
                    [128, 512], f32, tag="mm" if pool is None else "o",
                    name=f"psqk{m}_{nh}",
                )
                for c2 in range(2):
                    nc.tensor.matmul(
                        ps[:, :],
                        lhsT=wqk_sb[:, m, 2 * c2 : 2 * c2 + 2, 1, :],
                        rhs=x_sb[:, 4 + 2 * c2 : 6 + 2 * c2, 0, nh * 512 : (nh + 1) * 512],
                        start=(c2 == 0),
                        stop=False,
                        perf_mode=DR,
                    )
                for c in range(4):
                    nc.tensor.matmul(
                        ps[:, :],
                        lhsT=wqk_sb[:, m, c, :, :],
                        rhs=x_sb[:, 4 + c, :, nh * 512 : (nh + 1) * 512],
                        start=False,
                        stop=(c == 3),
                        perf_mode=DR,
                    )
                if on_act:
                    # ACT is idle before the first exp; using it for every
                    # other early copy doubles the psum recycle rate
                    nc.scalar.activation(
                        qkT[:, m, nh * 512 : (nh + 1) * 512],
                        ps[:, :],
                        Ident,
                        bias=bqk_col[:, m : m + 1],
                        scale=INV,
                    )
                else:
                    nc.vector.tensor_scalar(
                        qkT[:, m, nh * 512 : (nh + 1) * 512],
                        ps[:, :],
                        INV,
                        bqk_col[:, m : m + 1],
                        op0=mult,
                        op1=add,
                    )

            # ---------- v projection (same 3-term fp8 DR scheme) ------------
            v_aug = pp.tile([128, NT, HEADS * VSTRIDE], bf16)
            # only the per-head ones columns need initializing (col 128 of
            # each VSTRIDE block); the data columns are written by vproj
            ones_cols = v_aug.rearrange("p t (h w) -> p t h w", w=VSTRIDE)[
                :, :, :, 128:129
            ]
            nc.vector.memset(ones_cols, 1.0)
            out_sb = pp.tile([128, NT, D], bf16)
            # outT reuses x_sb's slot: x is dead once the v-projection is
            # done.  fp8 hi-first planes: the y projection runs as 3-term
            # DoubleRow like the other projections.
            outT = pp.tile([128, DC, 2, N], fp8, tag="xslot", name="outT")
            exps = {}

            def vproj_step(i):
                # i in [0, 16): t-tile i%8, output half i//8
                t, nh = i % NT, i // NT
                ps = ps_mm.tile([128, 512], f32, tag="mm", name=f"psv{t}_{nh}")
                for c2 in range(4):
                    nc.tensor.matmul(
                        ps[:, :],
                        lhsT=x_sb[:, 2 * c2 : 2 * c2 + 2, 0, t * 128 : (t + 1) * 128],
                        rhs=wv_sb[:, 2 * c2 : 2 * c2 + 2, 1, nh * 512 : (nh + 1) * 512],
                        start=(c2 == 0),
                        stop=False,
                        perf_mode=DR,
                    )
                for c in range(DC):
                    nc.tensor.matmul(
                        ps[:, :],
                        lhsT=x_sb[:, c, :, t * 128 : (t + 1) * 128],
                        rhs=wv_sb[:, c, :, nh * 512 : (nh + 1) * 512],
                        start=False,
                        stop=(c == DC - 1),
                        perf_mode=DR,
                    )
                dst = v_aug[:, t, nh * 4 * VSTRIDE : (nh + 1) * 4 * VSTRIDE]
                dst = dst.rearrange("p (h w) -> p h w", w=VSTRIDE)[:, :, 0:128]
                nc.vector.tensor_scalar(
                    dst,
                    ps[:, :].rearrange("p (h w) -> p h w", w=128),
                    INV,
                    None,
                    op0=mult,
                )

            # ---------- attention (bf16, as before) ------------------------
            def qkt_step(pair, jt):
                h0, h1 = 2 * pair, 2 * pair + 1
                pss = {
                    h: ps_s.tile([128, N], f32, tag="s", name=f"psS{h}_{jt}")
                    for h in (h0, h1)
                }
                for nh in range(2):
                    for h in (h0, h1):
                        pr = (h % 2) * 64
                        nc.tensor.matmul(
                            pss[h][:, nh * 512 : (nh + 1) * 512],
                            lhsT=qkT[pr : pr + 64, 2 * pair + 1, jt * 128 : (jt + 1) * 128],
                            rhs=qkT[pr : pr + 64, 2 * pair, nh * 512 : (nh + 1) * 512],
                            start=True,
                            stop=True,
                        )
                for h in (h0, h1):
                    nc.scalar.activation(
                        exps[h][:, jt, :], pss[h][:, :], Exp, scale=SCALE
                    )

            def pv_step(pair, s):
                # s in [0, 16): head pair*2 + s//8, i-tile s%8
                h, it = 2 * pair + s // NT, s % NT
                pso = ps_o.tile([128, 129], f32, tag="o", name=f"psO{h}_{it}")
                for jc in range(NT):
                    nc.tensor.matmul(
                        pso[:, :],
                        lhsT=exps[h][:, jc, it * 128 : (it + 1) * 128],
                        rhs=v_aug[:, jc, h * VSTRIDE : h * VSTRIDE + 129],
                        start=(jc == 0),
                        stop=(jc == NT - 1),
                    )
                recip = sp.tile([128, 1], f32, tag="recip", name=f"rc{h}_{it}")
                nc.vector.reciprocal(recip[:, :], pso[:, 128:129])
                # normalize AND add bv in one op: out = pso*recip + bv
                nc.vector.scalar_tensor_tensor(
                    out_sb[:, it, h * DV : (h + 1) * DV],
                    pso[:, 0:DV],
                    recip[:, :],
                    bv_bc[:, h * DV : (h + 1) * DV],
                    op0=mult,
                    op1=add,
                )

            def outT_step(hp, it, hi_on_act=False):
                # transpose head pair hp (chunks 2hp, 2hp+1) of i-tile `it`
                # (bv was already folded in at the PV divide), then emit the
                # fp8 hi/lo pair for the DoubleRow y projection:
                #   hi = fp8(out^T * 4)        (DVE, or ACT when exp is done)
                #   lo = out^T * 4 - hi        (DVE, fp8 out)
                pst = ps_mm.tile([128, 2, 128], bf16, tag="mm", name=f"psoT{hp}_{it}")
                for k in range(2):
                    c = 2 * hp + k
                    nc.tensor.transpose(
                        pst[:, k, :],
                        out_sb[:, it, c * 128 : (c + 1) * 128],
                        ident[:, :],
                    )
                hi = outT[:, 2 * hp : 2 * hp + 2, 0, it * 128 : (it + 1) * 128]
                lo = outT[:, 2 * hp : 2 * hp + 2, 1, it * 128 : (it + 1) * 128]
                if hi_on_act:
                    nc.scalar.activation(hi, pst[:, :, :], Ident, scale=SOUT)
                else:
                    nc.vector.tensor_scalar(hi, pst[:, :, :], SOUT, None, op0=mult)
                nc.vector.scalar_tensor_tensor(
                    lo, pst[:, :, :], SOUT, hi, op0=mult, op1=subtract
                )

            # prologue: the 4 qkT steps that gate pair 0; (1,1) is only
            # needed by jt 4-7 but its matmuls usefully cover the DVE copy
            # latency of the first three steps.
            qkt_proj_step(0, 0)
            qkt_proj_step(1, 0)
            # the last two prologue steps borrow the (still idle) PV psum
            # pool so all four steps have distinct psum tiles
            qkt_proj_step(0, 1, pool=ps_o)
            qkt_proj_step(1, 1, pool=ps_o)

            QKT_REST = [(2, 0), (3, 0), (2, 1), (3, 1), (4, 0), (5, 0),
                        (4, 1), (5, 1), (6, 0), (7, 0), (6, 1), (7, 1)]
            for pair in range(HEADS // 2):
                h0, h1 = 2 * pair, 2 * pair + 1
                exps[h0] = ep.tile([128, NT, N], bf16, tag="expS", name=f"eS{h0}")
                exps[h1] = ep.tile([128, NT, N], bf16, tag="expS", name=f"eS{h1}")
                for jt in range(NT):
                    qkt_step(pair, jt)
                    if pair == 0:
                        # jt 0-5: remaining qkT-projection steps; jt 6-7:
                        # first two v-projection steps (x embed + Wv have
                        # landed by then)
                        if jt < 6:
                            # odd steps borrow the PV psum pool (idle until
                            # pair 1) to relax the mm-pool rotation
                            qkt_proj_step(*QKT_REST[2 * jt])
                            qkt_proj_step(*QKT_REST[2 * jt + 1], pool=ps_o)
                        else:
                            vproj_step(jt - 6)
                    elif pair == 1:
                        if jt < 3:
                            vproj_step(2 + 2 * jt)
                            vproj_step(3 + 2 * jt)
                        else:
                            for q in range(3):
                                s = 3 * (jt - 3) + q
                                if s < 16:
                                    pv_step(0, s)
                        if jt == 7:
                            pv_step(0, 15)
                    elif pair == 2:
                        if jt < 4:
                            vproj_step(8 + 2 * jt)
                            vproj_step(9 + 2 * jt)
                        else:
                            for q in range(4):
                                pv_step(1, 4 * (jt - 4) + q)
                            # heads 0-1 (PV-0 done in pair 1); only after the
                            # last vproj: outT shares x_sb's SBUF slot
                            outT_step(0, 2 * (jt - 4))
                            outT_step(0, 2 * (jt - 4) + 1)
                    else:
                        pv_step(pair - 1, 2 * jt)
                        pv_step(pair - 1, 2 * jt + 1)
                        outT_step(1, jt)  # heads 2-3: PV-1 done in pair 2

            # ---------- tail: PV-3, outT heads 4-7, and y, pipelined per
            # i-tile: y(mt) only waits on the short hp2/hp3 chains of its own
            # tile.  exp is done, so the fp8 hi extraction runs on ACT here.
            def _emit_y(mt, nh):
                y_tile = yp.tile([128, 512], f32, tag="y", name=f"y{mt}_{nh}")
                # the very last tile is computed/stored as a 384-col piece
                # followed by a 128-col piece: the final copy+store chain
                # after the last matmul covers only 128 columns
                bounds = (0, 384, 512) if (mt == NT - 1 and nh == 1) else (0, 512)
                for sphalf in range(len(bounds) - 1):
                    w = bounds[sphalf + 1] - bounds[sphalf]
                    ps = ps_s.tile(
                        [128, w], f32, tag="s", name=f"psy{mt}_{nh}_{sphalf}"
                    )
                    sl = slice(bounds[sphalf], bounds[sphalf + 1])
                    dsl = slice(nh * 512 + bounds[sphalf], nh * 512 + bounds[sphalf + 1])
                    for c2 in range(4):
                        nc.tensor.matmul(
                            ps[:, :],
                            lhsT=outT[:, 2 * c2 : 2 * c2 + 2, 0, mt * 128 : (mt + 1) * 128],
                            rhs=wo_sb[:, 2 * c2 : 2 * c2 + 2, 1, dsl],
                            start=(c2 == 0),
                            stop=False,
                            perf_mode=DR,
                        )
                    for kc in range(DC):
                        nc.tensor.matmul(
                            ps[:, :],
                            lhsT=outT[:, kc, :, mt * 128 : (mt + 1) * 128],
                            rhs=wo_sb[:, kc, :, dsl],
                            start=False,
                            stop=(kc == DC - 1),
                            perf_mode=DR,
                        )
                    nc.vector.scalar_tensor_tensor(
                        y_tile[:, sl],
                        ps[:, :],
                        INV_Y,
                        bo_bc[:, dsl],
                        op0=mult,
                        op1=add,
                    )
                    eng = nc.scalar if (len(bounds) == 3 and sphalf == 0) else nc.sync
                    eng.dma_start(
                        out=y_d[mt * 128 : (mt + 1) * 128, dsl],
                        in_=y_tile[:, sl],
                    )

            for mtl in range(NT + 1):
                if mtl < NT:
                    # outT(2, mtl) first: it needs only PV-2 (done in pair
                    # 3), so it fills the wait for the last exps of pair 3
                    outT_step(2, mtl, hi_on_act=True)
                    pv_step(3, mtl)        # head 6, i-tile mtl
                    pv_step(3, 8 + mtl)    # head 7, i-tile mtl
                    outT_step(3, mtl, hi_on_act=True)
                if mtl < 1:
                    continue
                mt = mtl - 1
                _emit_y(mt, 0)
                _emit_y(mt, 1)

    nc.finalize()
    return nc


def _get_compiled():
    global _COMPILED
    if _COMPILED is None:
        _COMPILED = _build()
    return _COMPILED


def _split8(a, s):
    """fp8 hi/lo residual pair of (s * a); returned as fp8 arrays."""
    hi = (a * s).astype(E4NP)
    lo = (np.asarray(a * s, np.float32) - hi.astype(np.float32)).astype(E4NP)
    return hi, lo


def _prep_shared(inputs):
    Wqk = np.asarray(inputs["Wqk"], np.float32)
    bqk = np.asarray(inputs["bqk"], np.float32)
    Wv = np.asarray(inputs["Wv"], np.float32)
    bv = np.asarray(inputs["bv"], np.float32)
    Wo = np.asarray(inputs["Wo"], np.float32)
    bo = np.asarray(inputs["bo"], np.float32)

    # permute Wqk column blocks pair-major
    blocks = [Wqk[:, m * 128 : (m + 1) * 128] for m in _MPERM]
    wqk_r = np.stack(blocks, axis=0)                     # [8, 512, 128]
    bqk_r = np.concatenate([bqk[m * 128 : (m + 1) * 128] for m in _MPERM])

    def tile_w(w, kchunks):
        arr = w.reshape(kchunks, 128, w.shape[1])        # [c, p, n]
        hi, lo = _split8(arr, SW)
        # lo-first planes for the weight side
        return np.ascontiguousarray(
            np.stack([lo, hi], axis=2).transpose(1, 0, 2, 3)
        )                                                # [p, c, 2, n]

    # wqk m-block major: [p, m, c, 2, 128], lo-first planes
    arr = wqk_r.reshape(8, 4, 128, 128)                  # [m, c, p, col]
    hi, lo = _split8(arr, SW)
    wqk_dev = np.ascontiguousarray(
        np.stack([lo, hi], axis=3).transpose(2, 0, 1, 3, 4)
    )                                                    # [p, m, c, 2, col]

    bias = np.concatenate(
        [bqk_r.reshape(8, 128).T, bv.reshape(8, 128).T], axis=1
    )                                                    # [128, 16]
    shared = {
        "wqk": wqk_dev,
        "wv": tile_w(Wv, 8),
        "wo": tile_w(Wo, 8),
        "biasc": np.ascontiguousarray(bias),
        "bob": np.ascontiguousarray(
            np.tile(np.concatenate([bo, bv]), (128, 1)).astype(BFNP)
        ),
    }
    return shared


def _prep_x(xb):
    """One batch element -> xs [128, 8, 2, 1024] fp8 (hi-first)."""
    xt = np.asarray(xb, np.float32).T                    # [feat, tok]
    arr = xt.reshape(DC, 128, N)                         # [c, p, t]
    hi, lo = _split8(arr, SX)
    return np.ascontiguousarray(np.stack([hi, lo], axis=2).transpose(1, 0, 2, 3))


def _prep_in_maps(inputs):
    shared = _prep_shared(inputs)
    x = np.asarray(inputs["x"], np.float32)
    return [{"xs": _prep_x(x[b]), **shared} for b in range(B)]


def _run(inputs: dict, trace: bool = False):
    from concourse.bass_utils import run_bass_kernel_spmd

    nc = _get_compiled()
    in_maps = _prep_in_maps(inputs)
    res = run_bass_kernel_spmd(nc, in_maps, core_ids=list(range(B)), trace=trace)
    y = np.stack([res.results[b]["y"] for b in range(B)], axis=0)
    return y, res


def kernel(**inputs) -> np.ndarray:
    y, _ = _run(inputs, trace=False)
    return y
